# revision 13
# baseline (speedup 1.0000x reference)
"""Bahdanau attention kernel for 8 Trainium2 NeuronCores (v2: fp8 DoubleRow).

Problem shapes (hardcoded): hidden [2, 32, 1024], encoder_outputs [32, 2048, 1024],
Wq/Wk [1024, 1024], bq/bk/wv [1024], bv scalar. Output [32, 1, 1024].

Sharding: data-parallel over batch B=32 -> 4 batches per core, weights replicated.
bv is dropped entirely (softmax is invariant to constant shifts).

Structure vs the bf16 baseline:
- Weights are pre-transposed on the host (pure permutation, no flops) into the
  [p, t, c, n] tile layout the PE wants, so the on-device setup is just DMAs +
  casts: no fp32 PE transposes and no weight XBARs.
- The K-projection runs in fp8e4 DoubleRow (2 contraction rows/cycle): Wk is
  scaled x64 on device into e4m3 normal range; the exact 1/64 is folded into
  the tanh activation's input scale. Same trick for wv in the scores matmul
  (undone in the exp scale).
- enc is loaded once per 512-row chunk with a casting SWDGE DMA (fp32->bf16 in
  the DMA engine), XBAR-transposed to [h, s] bf16, then downcast to fp8 for the
  DoubleRow K-proj; the natural-layout copy is downcast to fp8 for the final
  attn @ enc einsum (also DoubleRow over s-pairs).
- Staging is chunk-granular with a deep prefetch window so the PE never waits
  for enc, and the per-chunk epilogue (attn transpose + einsum) is emitted one
  chunk late so exp latency hides under the next chunk's matmuls.
"""

from contextlib import ExitStack

import numpy as np

import concourse.bacc as bacc
import concourse.bass as bass
import concourse.mybir as mybir
import concourse.tile as tile
from concourse.bass_utils import run_bass_kernel_spmd

B, S, H = 32, 2048, 1024
NCORES = 8
BPC = B // NCORES  # 4 batches per core
F32 = mybir.dt.float32
BF16 = mybir.dt.bfloat16
F8 = mybir.dt.float8e4
HT = H // 128  # 8 chunks of 128 along h or o
SC = S // 512  # 4 s-chunks of 512 per batch
NCH = BPC * SC  # 16 chunks total per core
PREFETCH = 6  # chunks staged ahead of compute
WSCALE = 64.0  # fp8 pre-scale for Wk and wv (exact power of two)
Tanh = mybir.ActivationFunctionType.Tanh
Exp = mybir.ActivationFunctionType.Exp
X = mybir.AxisListType.X
DR = mybir.MatmulPerfMode.DoubleRow

ts = bass.ts


def build_program():
    nc = bacc.Bacc("TRN2", target_bir_lowering=False, debug=False)

    # hbw: host-packed [p, c, col] with cols 0-3 = hidden^T, 4 = bq^T, 5 = bk^T,
    # 6 = wv^T (value at [p, c, col] corresponds to h = 128c + p).
    hbw_d = nc.dram_tensor("hbw", [128, HT, 7], F32, kind="ExternalInput")
    enc_d = nc.dram_tensor("enc", [BPC, S, H], F32, kind="ExternalInput")
    # wkt4/wqt4: host-packed W^T tiles: [p, t, c, n] = W[128t + n, 128c + p].
    wkt4_d = nc.dram_tensor("wkt4", [128, HT, HT, 128], F32, kind="ExternalInput")
    wqt4_d = nc.dram_tensor("wqt4", [128, HT, HT, 128], F32, kind="ExternalInput")
    out_d = nc.dram_tensor("out", [BPC, 1, H], F32, kind="ExternalOutput")

    with tile.TileContext(nc) as tc, ExitStack() as ctx:
        consts = ctx.enter_context(tc.tile_pool(name="consts", bufs=1))
        tp = ctx.enter_context(tc.tile_pool(name="tp", bufs=2, space="PSUM"))
        kp = ctx.enter_context(tc.tile_pool(name="kp", bufs=4, space="PSUM"))
        vp = ctx.enter_context(tc.tile_pool(name="vp", bufs=2, space="PSUM"))

        encnat = ctx.enter_context(tc.tile_pool(name="encnat", bufs=2))
        ebnat = ctx.enter_context(tc.tile_pool(name="ebnat", bufs=PREFETCH + 2))
        encTbf = ctx.enter_context(tc.tile_pool(name="encTbf", bufs=3))
        encTp = ctx.enter_context(tc.tile_pool(name="encTp", bufs=PREFETCH + 1))
        eTp = ctx.enter_context(tc.tile_pool(name="eTp", bufs=2))
        batch = ctx.enter_context(tc.tile_pool(name="batch", bufs=1))
        setup = ctx.enter_context(tc.tile_pool(name="setup", bufs=2))

        ones_bf = consts.tile([1, 128], BF16, tag="ones")
        nc.vector.memset(ones_bf[:], 1.0)

        # ---- enc chunk staging ----
        # All loads go through HWDGE (scalar ring): the Tile scheduler
        # serializes SWDGE (gpsimd) DMAs globally against in-flight XBAR
        # transposes, which turns SWDGE staging into a ~20us/chunk serial
        # chain. The f32->bf16 downcast runs on the otherwise-idle gpsimd
        # engine; the bf16 natural tile feeds both the XBAR and the einsum,
        # and only the K-proj operand drops to fp8 (on DVE).
        def stage_chunk(b, j):
            en = encnat.tile([128, 4, H], F32, tag="encnat")
            nc.scalar.dma_start(
                en[:], enc_d[b, ts(j, 512), :].rearrange("(u p) h -> p u h", p=128)
            )
            eb = ebnat.tile([128, 4, H], BF16, tag="ebnat")
            nc.gpsimd.tensor_copy(eb[:], en[:])
            etbf = encTbf.tile([128, HT, 4, 128], BF16, tag="etbf")
            for u in range(4):
                nc.sync.dma_start_transpose(etbf[:, :, u, :], eb[:, u, :])
            etf8 = encTp.tile([128, HT, 4, 128], F8, tag="etf8")
            nc.vector.tensor_copy(etf8[:], etbf[:])
            return eb, etf8

        # ---- stage the first chunks + weight DMAs up front ----
        order = [(b, j) for b in range(BPC) for j in range(SC)]
        staged = {}
        for k in range(min(2, NCH)):
            staged[k] = stage_chunk(*order[k])

        # hbw: biases + hidden^T + wv^T (tiny, first on the scalar HWDGE queue)
        hbw = setup.tile([128, HT, 7], F32, tag="hbw", bufs=1)
        nc.scalar.dma_start(hbw[:], hbw_d[:])
        bsum = consts.tile([128, HT, 1], F32, tag="bsum")
        nc.vector.tensor_add(bsum[:], hbw[:, :, 4:5], hbw[:, :, 5:6])
        wvT_f8 = consts.tile([128, HT, 16], F8, tag="wvTf8")
        nc.scalar.mul(wvT_f8[:, :, 0:1], hbw[:, :, 6:7], WSCALE)
        hidT_bf = consts.tile([128, HT, BPC], BF16, tag="hidTbf")
        nc.vector.tensor_copy(hidT_bf[:], hbw[:, :, 0:BPC])

        # Wk/Wq tiles interleaved on the scalar HWDGE queue so tile t of both
        # is usable early: kproj output-tile i only needs Wk tile i, and the
        # tanh bias for tile i only needs Wq tile i.
        wkT4_f8 = consts.tile([128, HT, HT, 128], F8, tag="wkT4f8")
        qkb = consts.tile([128, HT, BPC], F32, tag="qkb")
        for t in range(HT):
            wkf = setup.tile([128, HT, 128], F32, tag="wkf")
            nc.scalar.dma_start(wkf[:], wkt4_d[:, t, :, :])
            nc.scalar.mul(wkT4_f8[:, t, :, :], wkf[:], WSCALE)
            wqf = setup.tile([128, HT, 128], F32, tag="wqf")
            nc.scalar.dma_start(wqf[:], wqt4_d[:, t, :, :])
            wqbf = setup.tile([128, HT, 128], BF16, tag="wqbf")
            nc.vector.tensor_copy(wqbf[:], wqf[:])
            pq = kp.tile([128, BPC], F32, tag="kp")
            for c in range(HT):
                nc.tensor.matmul(
                    pq[:],
                    wqbf[:, c, :],
                    hidT_bf[:, c, :],
                    start=(c == 0),
                    stop=(c == HT - 1),
                )
            nc.vector.tensor_scalar_add(qkb[:, t, :], pq[:], bsum[:, t, :])

        # remaining prefetch window
        for k in range(2, min(PREFETCH, NCH)):
            staged[k] = stage_chunk(*order[k])

        # ---- per-chunk compute pieces ----
        state = {}

        def kproj_scores(b, j, etf8):
            eT = eTp.tile([128, HT, 512], F8, tag="eT")
            for i in range(HT):
                pk = kp.tile([128, 512], F32, tag="kp")
                for dc in range(HT // 2):
                    nc.tensor.matmul(
                        pk[:],
                        wkT4_f8[:, i, 2 * dc : 2 * dc + 2, :],
                        etf8[:, 2 * dc : 2 * dc + 2, :, :],
                        start=(dc == 0),
                        stop=(dc == HT // 2 - 1),
                        perf_mode=DR,
                    )
                nc.scalar.activation(
                    eT[:, i, :], pk[:], Tanh, bias=qkb[:, i, b : b + 1],
                    scale=1.0 / WSCALE,
                )
            ps = kp.tile([1, 512], F32, tag="kp")
            for dc in range(HT // 2):
                nc.tensor.matmul(
                    ps[:],
                    wvT_f8[:, 2 * dc : 2 * dc + 2, 0:1],
                    eT[:, 2 * dc : 2 * dc + 2, :],
                    start=(dc == 0),
                    stop=(dc == HT // 2 - 1),
                    perf_mode=DR,
                )
            expj = batch.tile([1, 512], BF16, tag="expj", bufs=3)
            nc.scalar.activation(
                expj[:], ps[:], Exp, scale=1.0 / WSCALE,
                accum_out=state["ssum4"][0:1, j : j + 1],
            )
            return expj

        def epilogue(j, expj, eb, po):
            # attn weights -> [s(part), u] columns (bf16), then bf16 einsum
            atT = batch.tile([128, 4], BF16, tag="atT", bufs=2)
            for u in range(4):
                pa = tp.tile([128, 1], BF16, tag="tp")
                nc.tensor.transpose(
                    pa[:], expj[0:1, ts(u, 128)], ones_bf[0:1, 0:1]
                )
                nc.scalar.copy(atT[:, u : u + 1], pa[:])
            for hc in range(2):
                for u in range(4):
                    nc.tensor.matmul(
                        po[hc][:],
                        atT[:, u : u + 1],
                        eb[:, u, ts(hc, 512)],
                        start=(j == 0 and u == 0),
                        stop=(j == SC - 1 and u == 3),
                    )

        def batch_start():
            po0 = vp.tile([1, 512], F32, tag="vp")
            po1 = vp.tile([1, 512], F32, tag="vp")
            ssum4 = batch.tile([1, SC], F32, tag="ssum4", bufs=2)
            state["po"] = (po0, po1)
            state["ssum4"] = ssum4

        def batch_finalize(b, ssum4, po):
            ssum = batch.tile([1, 1], F32, tag="ssum", bufs=2)
            nc.vector.reduce_sum(ssum[:], ssum4[:], axis=X)
            inv = batch.tile([1, 1], F32, tag="inv", bufs=2)
            nc.vector.reciprocal(inv[:], ssum[:])
            outb = batch.tile([1, H], F32, tag="outb", bufs=2)
            nc.vector.tensor_scalar_mul(outb[0:1, ts(0, 512)], po[0][:], inv[0:1, 0:1])
            nc.vector.tensor_scalar_mul(outb[0:1, ts(1, 512)], po[1][:], inv[0:1, 0:1])
            nc.scalar.dma_start(out_d[b], outb[:])

        # ---- main loop: epilogue delayed one chunk so exp hides under matmuls ----
        pending = None  # (b, j, expj, ebf8, ssum4, po)
        for idx, (b, j) in enumerate(order):
            if idx + PREFETCH < NCH:
                staged[idx + PREFETCH] = stage_chunk(*order[idx + PREFETCH])
            ebf8, etf8 = staged.pop(idx)
            if j == 0:
                # finalize the previous batch before its PSUM pair is reused
                if pending is not None:
                    pb, pj, pexp, pebf8, pssum4, ppo = pending
                    epilogue(pj, pexp, pebf8, ppo)
                    batch_finalize(pb, pssum4, ppo)
                    pending = None
                batch_start()
            expj = kproj_scores(b, j, etf8)
            if pending is not None:
                pb, pj, pexp, pebf8, pssum4, ppo = pending
                epilogue(pj, pexp, pebf8, ppo)
            pending = (b, j, expj, ebf8, state["ssum4"], state["po"])
        pb, pj, pexp, pebf8, pssum4, ppo = pending
        epilogue(pj, pexp, pebf8, ppo)
        batch_finalize(pb, pssum4, ppo)

    nc.compile()
    return nc


_CACHED_NC = None


def _get_nc():
    global _CACHED_NC
    if _CACHED_NC is None:
        _CACHED_NC = build_program()
    return _CACHED_NC


def make_in_maps(hidden, encoder_outputs, Wq, bq, Wk, bk, wv):
    """Host-side packing: pure slicing/transposition only (no arithmetic)."""
    hid_last = np.asarray(hidden, np.float32)[-1]  # [32, H]
    enc = np.asarray(encoder_outputs, np.float32)
    Wq = np.asarray(Wq, np.float32)
    Wk = np.asarray(Wk, np.float32)
    bq = np.asarray(bq, np.float32).reshape(H)
    bk = np.asarray(bk, np.float32).reshape(H)
    wv = np.asarray(wv, np.float32).reshape(H)

    # [p, t, c, n] = W[128t + n, 128c + p]
    def pack_wT(W):
        return np.ascontiguousarray(
            W.reshape(HT, 128, HT, 128).transpose(3, 0, 2, 1)
        )

    wkt4 = pack_wT(Wk)
    wqt4 = pack_wT(Wq)

    def pack_hbw(hid_slice):
        hbw = np.empty((128, HT, 7), np.float32)
        hbw[:, :, 0:BPC] = hid_slice.T.reshape(HT, 128, BPC).transpose(1, 0, 2)
        hbw[:, :, 4] = bq.reshape(HT, 128).T
        hbw[:, :, 5] = bk.reshape(HT, 128).T
        hbw[:, :, 6] = wv.reshape(HT, 128).T
        return hbw

    in_maps = []
    for c in range(NCORES):
        sl = slice(c * BPC, (c + 1) * BPC)
        in_maps.append(
            {
                "hbw": pack_hbw(hid_last[sl]),
                "enc": np.ascontiguousarray(enc[sl]),
                "wkt4": wkt4,
                "wqt4": wqt4,
            }
        )
    return in_maps


def run(inputs, trace=False):
    """Run on hardware; returns (output [32,1,1024], BassKernelResults)."""
    nc = _get_nc()
    in_maps = make_in_maps(
        inputs["hidden"],
        inputs["encoder_outputs"],
        inputs["Wq"],
        inputs["bq"],
        inputs["Wk"],
        inputs["bk"],
        inputs["wv"],
    )
    res = run_bass_kernel_spmd(nc, in_maps, list(range(NCORES)), trace=trace)
    out = np.concatenate([res.results[c]["out"] for c in range(NCORES)], axis=0)
    return out.reshape(B, 1, H).astype(np.float32), res


def kernel(hidden, encoder_outputs, Wq, bq, Wk, bk, wv, bv):
    out, _ = run(
        {
            "hidden": hidden,
            "encoder_outputs": encoder_outputs,
            "Wq": Wq,
            "bq": bq,
            "Wk": Wk,
            "bk": bk,
            "wv": wv,
        }
    )
    return out


# revision 14
# speedup vs baseline: 1.1472x; 1.1472x over previous
"""Bahdanau attention kernel for 8 Trainium2 NeuronCores (v2: fp8 DoubleRow).

Problem shapes (hardcoded): hidden [2, 32, 1024], encoder_outputs [32, 2048, 1024],
Wq/Wk [1024, 1024], bq/bk/wv [1024], bv scalar. Output [32, 1, 1024].

Sharding: data-parallel over batch B=32 -> 4 batches per core, weights replicated.
bv is dropped entirely (softmax is invariant to constant shifts).

Structure vs the bf16 baseline:
- Weights are pre-transposed on the host (pure permutation, no flops) into the
  [p, t, c, n] tile layout the PE wants, so the on-device setup is just DMAs +
  casts: no fp32 PE transposes and no weight XBARs.
- The K-projection runs in fp8e4 DoubleRow (2 contraction rows/cycle): Wk is
  scaled x64 on device into e4m3 normal range; the exact 1/64 is folded into
  the tanh activation's input scale. Same trick for wv in the scores matmul
  (undone in the exp scale).
- enc is loaded once per 512-row chunk with a casting SWDGE DMA (fp32->bf16 in
  the DMA engine), XBAR-transposed to [h, s] bf16, then downcast to fp8 for the
  DoubleRow K-proj; the natural-layout copy is downcast to fp8 for the final
  attn @ enc einsum (also DoubleRow over s-pairs).
- Staging is chunk-granular with a deep prefetch window so the PE never waits
  for enc, and the per-chunk epilogue (attn transpose + einsum) is emitted one
  chunk late so exp latency hides under the next chunk's matmuls.
"""

from contextlib import ExitStack

import numpy as np

import concourse.bacc as bacc
import concourse.bass as bass
import concourse.mybir as mybir
import concourse.tile as tile
from concourse.bass_utils import run_bass_kernel_spmd

B, S, H = 32, 2048, 1024
NCORES = 8
BPC = B // NCORES  # 4 batches per core
F32 = mybir.dt.float32
BF16 = mybir.dt.bfloat16
F8 = mybir.dt.float8e4
HT = H // 128  # 8 chunks of 128 along h or o
SC = S // 512  # 4 s-chunks of 512 per batch
NCH = BPC * SC  # 16 chunks total per core
PREFETCH = 6  # chunks staged ahead of compute
WSCALE = 64.0  # fp8 pre-scale for Wk and wv (exact power of two)
Tanh = mybir.ActivationFunctionType.Tanh
Exp = mybir.ActivationFunctionType.Exp
X = mybir.AxisListType.X
DR = mybir.MatmulPerfMode.DoubleRow

ts = bass.ts


def build_program():
    nc = bacc.Bacc("TRN2", target_bir_lowering=False, debug=False)

    # hbw: host-packed [p, c, col] with cols 0-3 = hidden^T, 4 = bq^T, 5 = bk^T,
    # 6 = wv^T (value at [p, c, col] corresponds to h = 128c + p).
    hbw_d = nc.dram_tensor("hbw", [128, HT, 7], F32, kind="ExternalInput")
    enc_d = nc.dram_tensor("enc", [BPC, S, H], F32, kind="ExternalInput")
    # wkt4/wqt4: host-packed W^T tiles: [p, t, c, n] = W[128t + n, 128c + p].
    wkt4_d = nc.dram_tensor("wkt4", [128, HT, HT, 128], F32, kind="ExternalInput")
    wqt4_d = nc.dram_tensor("wqt4", [128, HT, HT, 128], F32, kind="ExternalInput")
    out_d = nc.dram_tensor("out", [BPC, 1, H], F32, kind="ExternalOutput")

    with tile.TileContext(nc) as tc, ExitStack() as ctx:
        consts = ctx.enter_context(tc.tile_pool(name="consts", bufs=1))
        tp = ctx.enter_context(tc.tile_pool(name="tp", bufs=2, space="PSUM"))
        kp = ctx.enter_context(tc.tile_pool(name="kp", bufs=4, space="PSUM"))
        vp = ctx.enter_context(tc.tile_pool(name="vp", bufs=2, space="PSUM"))

        encnat = ctx.enter_context(tc.tile_pool(name="encnat", bufs=2))
        ebnat = ctx.enter_context(tc.tile_pool(name="ebnat", bufs=PREFETCH + 2))
        encTbf = ctx.enter_context(tc.tile_pool(name="encTbf", bufs=3))
        encTp = ctx.enter_context(tc.tile_pool(name="encTp", bufs=PREFETCH + 1))
        eTp = ctx.enter_context(tc.tile_pool(name="eTp", bufs=2))
        batch = ctx.enter_context(tc.tile_pool(name="batch", bufs=1))
        setup = ctx.enter_context(tc.tile_pool(name="setup", bufs=2))

        ones_bf = consts.tile([1, 128], BF16, tag="ones")
        nc.vector.memset(ones_bf[:], 1.0)

        # ---- enc chunk staging ----
        # All loads go through HWDGE (scalar ring): the Tile scheduler
        # serializes SWDGE (gpsimd) DMAs globally against in-flight XBAR
        # transposes, which turns SWDGE staging into a ~20us/chunk serial
        # chain. The f32->bf16 downcast runs on the otherwise-idle gpsimd
        # engine; the bf16 natural tile feeds both the XBAR and the einsum,
        # and only the K-proj operand drops to fp8 (on DVE).
        def stage_chunk(b, j):
            en = encnat.tile([128, 4, H], F32, tag="encnat")
            nc.scalar.dma_start(
                en[:], enc_d[b, ts(j, 512), :].rearrange("(u p) h -> p u h", p=128)
            )
            eb = ebnat.tile([128, 4, H], BF16, tag="ebnat")
            nc.vector.tensor_copy(eb[:], en[:])
            etbf = encTbf.tile([128, HT, 4, 128], BF16, tag="etbf")
            for u in range(4):
                nc.sync.dma_start_transpose(etbf[:, :, u, :], eb[:, u, :])
            etf8 = encTp.tile([128, HT, 4, 128], F8, tag="etf8")
            nc.vector.tensor_copy(etf8[:], etbf[:])
            return eb, etf8

        # ---- stage the first chunks + weight DMAs up front ----
        order = [(b, j) for b in range(BPC) for j in range(SC)]
        staged = {}
        for k in range(min(2, NCH)):
            staged[k] = stage_chunk(*order[k])

        # hbw: biases + hidden^T + wv^T (tiny, first on the scalar HWDGE queue)
        hbw = setup.tile([128, HT, 7], F32, tag="hbw", bufs=1)
        nc.scalar.dma_start(hbw[:], hbw_d[:])
        bsum = consts.tile([128, HT, 1], F32, tag="bsum")
        nc.vector.tensor_add(bsum[:], hbw[:, :, 4:5], hbw[:, :, 5:6])
        wvT_f8 = consts.tile([128, HT, 16], F8, tag="wvTf8")
        nc.scalar.mul(wvT_f8[:, :, 0:1], hbw[:, :, 6:7], WSCALE)
        hidT_bf = consts.tile([128, HT, BPC], BF16, tag="hidTbf")
        nc.vector.tensor_copy(hidT_bf[:], hbw[:, :, 0:BPC])

        # Wk/Wq tiles interleaved on the scalar HWDGE queue so tile t of both
        # is usable early: kproj output-tile i only needs Wk tile i, and the
        # tanh bias for tile i only needs Wq tile i.
        wkT4_f8 = consts.tile([128, HT, HT, 128], F8, tag="wkT4f8")
        qkb = consts.tile([128, HT, BPC], F32, tag="qkb")
        for t in range(HT):
            wkf = setup.tile([128, HT, 128], F32, tag="wkf")
            nc.scalar.dma_start(wkf[:], wkt4_d[:, t, :, :])
            nc.scalar.mul(wkT4_f8[:, t, :, :], wkf[:], WSCALE)
            wqf = setup.tile([128, HT, 128], F32, tag="wqf")
            nc.scalar.dma_start(wqf[:], wqt4_d[:, t, :, :])
            wqbf = setup.tile([128, HT, 128], BF16, tag="wqbf")
            nc.vector.tensor_copy(wqbf[:], wqf[:])
            pq = kp.tile([128, BPC], F32, tag="kp")
            for c in range(HT):
                nc.tensor.matmul(
                    pq[:],
                    wqbf[:, c, :],
                    hidT_bf[:, c, :],
                    start=(c == 0),
                    stop=(c == HT - 1),
                )
            nc.vector.tensor_scalar_add(qkb[:, t, :], pq[:], bsum[:, t, :])

        # remaining prefetch window
        for k in range(2, min(PREFETCH, NCH)):
            staged[k] = stage_chunk(*order[k])

        # ---- per-chunk compute pieces ----
        state = {}

        def kproj_scores(b, j, etf8):
            eT = eTp.tile([128, HT, 512], F8, tag="eT")
            for i in range(HT):
                pk = kp.tile([128, 512], F32, tag="kp")
                for dc in range(HT // 2):
                    nc.tensor.matmul(
                        pk[:],
                        wkT4_f8[:, i, 2 * dc : 2 * dc + 2, :],
                        etf8[:, 2 * dc : 2 * dc + 2, :, :],
                        start=(dc == 0),
                        stop=(dc == HT // 2 - 1),
                        perf_mode=DR,
                    )
                nc.scalar.activation(
                    eT[:, i, :], pk[:], Tanh, bias=qkb[:, i, b : b + 1],
                    scale=1.0 / WSCALE,
                )
            ps = kp.tile([1, 512], F32, tag="kp")
            for dc in range(HT // 2):
                nc.tensor.matmul(
                    ps[:],
                    wvT_f8[:, 2 * dc : 2 * dc + 2, 0:1],
                    eT[:, 2 * dc : 2 * dc + 2, :],
                    start=(dc == 0),
                    stop=(dc == HT // 2 - 1),
                    perf_mode=DR,
                )
            expj = batch.tile([1, 512], BF16, tag="expj", bufs=3)
            nc.scalar.activation(
                expj[:], ps[:], Exp, scale=1.0 / WSCALE,
                accum_out=state["ssum4"][0:1, j : j + 1],
            )
            return expj

        def epilogue(j, expj, eb, po):
            # attn weights -> [s(part), u] columns (bf16), then bf16 einsum
            atT = batch.tile([128, 4], BF16, tag="atT", bufs=2)
            for u in range(4):
                pa = tp.tile([128, 1], BF16, tag="tp")
                nc.tensor.transpose(
                    pa[:], expj[0:1, ts(u, 128)], ones_bf[0:1, 0:1]
                )
                nc.scalar.copy(atT[:, u : u + 1], pa[:])
            for hc in range(2):
                for u in range(4):
                    nc.tensor.matmul(
                        po[hc][:],
                        atT[:, u : u + 1],
                        eb[:, u, ts(hc, 512)],
                        start=(j == 0 and u == 0),
                        stop=(j == SC - 1 and u == 3),
                    )

        def batch_start():
            po0 = vp.tile([1, 512], F32, tag="vp")
            po1 = vp.tile([1, 512], F32, tag="vp")
            ssum4 = batch.tile([1, SC], F32, tag="ssum4", bufs=2)
            state["po"] = (po0, po1)
            state["ssum4"] = ssum4

        def batch_finalize(b, ssum4, po):
            ssum = batch.tile([1, 1], F32, tag="ssum", bufs=2)
            nc.vector.reduce_sum(ssum[:], ssum4[:], axis=X)
            inv = batch.tile([1, 1], F32, tag="inv", bufs=2)
            nc.vector.reciprocal(inv[:], ssum[:])
            outb = batch.tile([1, H], F32, tag="outb", bufs=2)
            nc.vector.tensor_scalar_mul(outb[0:1, ts(0, 512)], po[0][:], inv[0:1, 0:1])
            nc.vector.tensor_scalar_mul(outb[0:1, ts(1, 512)], po[1][:], inv[0:1, 0:1])
            nc.scalar.dma_start(out_d[b], outb[:])

        # ---- main loop: epilogue delayed one chunk so exp hides under matmuls ----
        pending = None  # (b, j, expj, ebf8, ssum4, po)
        for idx, (b, j) in enumerate(order):
            if idx + PREFETCH < NCH:
                staged[idx + PREFETCH] = stage_chunk(*order[idx + PREFETCH])
            ebf8, etf8 = staged.pop(idx)
            if j == 0:
                # finalize the previous batch before its PSUM pair is reused
                if pending is not None:
                    pb, pj, pexp, pebf8, pssum4, ppo = pending
                    epilogue(pj, pexp, pebf8, ppo)
                    batch_finalize(pb, pssum4, ppo)
                    pending = None
                batch_start()
            expj = kproj_scores(b, j, etf8)
            if pending is not None:
                pb, pj, pexp, pebf8, pssum4, ppo = pending
                epilogue(pj, pexp, pebf8, ppo)
            pending = (b, j, expj, ebf8, state["ssum4"], state["po"])
        pb, pj, pexp, pebf8, pssum4, ppo = pending
        epilogue(pj, pexp, pebf8, ppo)
        batch_finalize(pb, pssum4, ppo)

    nc.compile()
    return nc


_CACHED_NC = None


def _get_nc():
    global _CACHED_NC
    if _CACHED_NC is None:
        _CACHED_NC = build_program()
    return _CACHED_NC


def make_in_maps(hidden, encoder_outputs, Wq, bq, Wk, bk, wv):
    """Host-side packing: pure slicing/transposition only (no arithmetic)."""
    hid_last = np.asarray(hidden, np.float32)[-1]  # [32, H]
    enc = np.asarray(encoder_outputs, np.float32)
    Wq = np.asarray(Wq, np.float32)
    Wk = np.asarray(Wk, np.float32)
    bq = np.asarray(bq, np.float32).reshape(H)
    bk = np.asarray(bk, np.float32).reshape(H)
    wv = np.asarray(wv, np.float32).reshape(H)

    # [p, t, c, n] = W[128t + n, 128c + p]
    def pack_wT(W):
        return np.ascontiguousarray(
            W.reshape(HT, 128, HT, 128).transpose(3, 0, 2, 1)
        )

    wkt4 = pack_wT(Wk)
    wqt4 = pack_wT(Wq)

    def pack_hbw(hid_slice):
        hbw = np.empty((128, HT, 7), np.float32)
        hbw[:, :, 0:BPC] = hid_slice.T.reshape(HT, 128, BPC).transpose(1, 0, 2)
        hbw[:, :, 4] = bq.reshape(HT, 128).T
        hbw[:, :, 5] = bk.reshape(HT, 128).T
        hbw[:, :, 6] = wv.reshape(HT, 128).T
        return hbw

    in_maps = []
    for c in range(NCORES):
        sl = slice(c * BPC, (c + 1) * BPC)
        in_maps.append(
            {
                "hbw": pack_hbw(hid_last[sl]),
                "enc": np.ascontiguousarray(enc[sl]),
                "wkt4": wkt4,
                "wqt4": wqt4,
            }
        )
    return in_maps


def run(inputs, trace=False):
    """Run on hardware; returns (output [32,1,1024], BassKernelResults)."""
    nc = _get_nc()
    in_maps = make_in_maps(
        inputs["hidden"],
        inputs["encoder_outputs"],
        inputs["Wq"],
        inputs["bq"],
        inputs["Wk"],
        inputs["bk"],
        inputs["wv"],
    )
    res = run_bass_kernel_spmd(nc, in_maps, list(range(NCORES)), trace=trace)
    out = np.concatenate([res.results[c]["out"] for c in range(NCORES)], axis=0)
    return out.reshape(B, 1, H).astype(np.float32), res


def kernel(hidden, encoder_outputs, Wq, bq, Wk, bk, wv, bv):
    out, _ = run(
        {
            "hidden": hidden,
            "encoder_outputs": encoder_outputs,
            "Wq": Wq,
            "bq": bq,
            "Wk": Wk,
            "bk": bk,
            "wv": wv,
        }
    )
    return out
